# revision 11
# baseline (speedup 1.0000x reference)
# Trainium2 Bass kernel for nn_Decoder (LSTM decoder + GCN message passing).
#
# Strategy (8 NeuronCores, SPMD):
#   * Data-parallel over nodes N=10000 -> 1250 nodes/core for fc2 + LSTM +
#     projection. State kept feature-major ([H, nodes]) so every matmul is
#     PE-friendly with K=H=128 and no transposes.
#   * Algebraic rewrite: the GCN aggregation and fc3 are both linear, so
#     aggregate AFTER projecting features to NF=16:
#        x_hat[n,t] = sum_{e: dst=n} (dinv[src]*dinv[n]*mask[src] * hs[t,src]
#                     @ (W_gcn@W_fc3)) + (b_gcn@W_fc3 + b_fc3)
#     This shrinks the scatter/gather payload 8x (H=128 -> NF=16 per t).
#     All normalization (dinv[src]*dinv[dst]*mask[src]) is folded into the
#     densified adjacency on the host, so the device ships raw projections.
#   * Source space padded to 1280 slots/core (10240 global) so every source
#     block is a full, core-aligned 128 rows: the post-AllGather reshuffle
#     into the SBUF Y-table is 8 large strided DMAs instead of ~10k tiny
#     descriptors.
#   * Y table ([128, 80*192] fp16) is built from 3 AllGather slices shipped
#     at t=5/8/11 so only the last small slice sits on the critical path.
#   * The scatter-add over edges is a block-dense matmul: the host densifies
#     the normalized adjacency into 128x128 blocks A[dst_tile, src_blk] and
#     each core computes agg[dst_tile] = sum_sb A[dst_tile,sb].T @ Y[sb].
#   * LSTM tail (c update, tanh, h) runs in fp16 split into two node-halves
#     so the next step's matmuls start as soon as the first half lands.
import os
import numpy as np

import concourse.bass as bass
import concourse.bacc as bacc
import concourse.tile as tile
from concourse import mybir
from concourse import bass_utils

P = 128
N, T, NF, H, L, E = 10000, 12, 16, 128, 64, 160000
NCORES = 8
NCN = N // NCORES            # 1250 real nodes per core
NPAD = 1280                  # padded node slots per core (10 full blocks)
NTILES = NPAD // P           # 10 tiles per core
NSB = (NPAD * NCORES) // P   # 80 padded source blocks
CH = [(0, 512), (512, 512), (1024, NCN - 1024)]  # matmul chunks (<=512 psum)
HALVES = [(0, 512), (512, NCN - 512)]            # LSTM tail halves
TNF = T * NF                 # 192
SL = [(0, 64), (64, 64), (128, 48), (176, 16)]  # AllGather slices
SHIP_T = {3: 0, 7: 1, 10: 2}     # proj step -> slice to ship (last at end)

F32 = mybir.dt.float32
F16 = mybir.dt.float16

# gate q: 0=i, 1=f, 2=g, 3=o ; activation: sigmoid for i,f,o ; tanh for g
GATE_FUNCS = ["Sigmoid", "Sigmoid", "Tanh", "Sigmoid"]
GATE_ORDER = [1, 0, 2, 3]    # f first so c*f can start earliest

_BUILD_CACHE = {}
LAST_RESULTS = None  # BassKernelResults of the most recent run (for harness)


def _build():
    nc = bacc.Bacc("TRN2", target_bir_lowering=False, debug=False,
                   num_devices=NCORES)

    # ---------------- I/O declarations ----------------
    zT = nc.dram_tensor("zT", [L, NCN], F32, kind="ExternalInput")
    wfc2 = nc.dram_tensor("wfc2", [L, H], F32, kind="ExternalInput")
    b2 = nc.dram_tensor("b2", [P, 1], F32, kind="ExternalInput")
    wih = nc.dram_tensor("wih", [H, 4 * H], F16, kind="ExternalInput")
    whh = nc.dram_tensor("whh", [H, 4 * H], F16, kind="ExternalInput")
    bg = nc.dram_tensor("bg", [P, 4], F32, kind="ExternalInput")
    wcomb = nc.dram_tensor("wcomb", [H, NF], F16, kind="ExternalInput")
    bout = nc.dram_tensor("bout", [P, TNF], F32, kind="ExternalInput")
    # A-blocks, wave-major: row (w*NSB + sb)*128 + srel ; col k_loc*128 + drel
    ablk = nc.dram_tensor("ablk", [2 * NSB * P, 5 * P], F16,
                          kind="ExternalInput")
    xhat = nc.dram_tensor("xhat", [NCN, TNF], F32, kind="ExternalOutput")

    with tile.TileContext(nc) as tc:
        with tc.tile_pool(name="cpool", bufs=1) as cp, \
             tc.tile_pool(name="spool", bufs=1) as sp, \
             tc.tile_pool(name="dram", bufs=1, space="DRAM") as dp:

            # ---- constant loads ----
            zt_sb = cp.tile([L, NCN], F32)
            for off, sz in CH:
                nc.sync.dma_start(zt_sb[:, off:off + sz],
                                  zT[:, off:off + sz])
            wfc2_sb = cp.tile([L, H], F32)
            nc.sync.dma_start(wfc2_sb[:], wfc2[:])
            b2_sb = cp.tile([P, 1], F32)
            nc.sync.dma_start(b2_sb[:], b2[:])
            wih_sb = cp.tile([H, 4 * H], F16)
            nc.sync.dma_start(wih_sb[:], wih[:])
            whh_sb = cp.tile([H, 4 * H], F16)
            nc.sync.dma_start(whh_sb[:], whh[:])
            bg_sb = cp.tile([P, 4], F32)
            nc.sync.dma_start(bg_sb[:], bg[:])
            wcomb_sb = cp.tile([H, NF], F16)
            nc.sync.dma_start(wcomb_sb[:], wcomb[:])
            bout_sb = cp.tile([P, TNF], F32)
            nc.sync.dma_start(bout_sb[:], bout[:])

            # tiny warmup collective: absorbs launch skew + CC cold start
            wsrc = dp.tile([P, 4], F16, name="wsrc")
            wdst = dp.tile([NCORES * P, 4], F16, addr_space="Shared",
                           name="wdst")
            nc.gpsimd.collective_compute(
                "AllGather", mybir.AluOpType.bypass,
                replica_groups=[list(range(NCORES))],
                ins=[wsrc.opt()], outs=[wdst.opt()],
            )

            # per-step projections, node-major padded: [p, (ktile, t*16+f)]
            ysb3 = sp.tile([P, NTILES * TNF], F16, name="ysb3")
            nc.vector.memset(ysb3[:], 0.0)  # pad rows must stay finite zeros

            yshard_s = [dp.tile([P, NTILES * w], F16, name=f"yshard{i}")
                        for i, (c0, w) in enumerate(SL)]
            yfull_s = [dp.tile([NCORES * P, NTILES * w], F16,
                               addr_space="Shared", name=f"yfull{i}")
                       for i, (c0, w) in enumerate(SL)]

            # Y table: ytab[p, sb*192 + t*16 + f] = Y[sb*128+p, t*16+f]
            ytab = sp.tile([P, NSB * TNF], F16, name="ytab")

            # ---- hd = z @ W_fc2 + b_fc2 (feature-major: hdT [H, nodes]) ----
            hdT = sp.tile([H, NCN], F16)
            with tc.tile_pool(name="psI", bufs=2, space="PSUM") as psI:
                for off, sz in CH:
                    ph = psI.tile([P, 512], F32, tag="ph", bufs=2)
                    nc.tensor.matmul(out=ph[:, :sz], lhsT=wfc2_sb[:],
                                     rhs=zt_sb[:, off:off + sz],
                                     start=True, stop=True)
                    nc.scalar.activation(
                        out=hdT[:, off:off + sz], in_=ph[:, :sz],
                        func=mybir.ActivationFunctionType.Identity,
                        bias=b2_sb[:, :1])

            # ---- prefetch A-blocks early on the sync queue ----
            CHUNK = 8
            sb_chunks = [(s0, min(s0 + CHUNK, NSB))
                         for s0 in range(0, NSB, CHUNK)]
            wpC_pool = tc.tile_pool(name="wpC", bufs=1)
            wpC = wpC_pool.__enter__()
            abc_tiles = []
            for w in range(2):
                for (s0, s1) in sb_chunks:
                    nsb_c = s1 - s0
                    abc = wpC.tile([P, CHUNK * 5 * P], F16, tag="abc",
                                   name=f"abc_{w}_{s0}", bufs=6)
                    r0 = (w * NSB + s0) * P
                    r1 = (w * NSB + s1) * P
                    nc.sync.dma_start(
                        abc[:, :nsb_c * 5 * P].rearrange(
                            "p (sb d) -> p sb d", d=5 * P),
                        ablk[r0:r1, :].rearrange("(sb p) d -> p sb d", p=P))
                    abc_tiles.append(abc)

            # ---- LSTM (T steps, feature-major state) ----
            c_half = [sp.tile([P, sz], F16, name=f"c{j}")
                      for j, (off, sz) in enumerate(HALVES)]
            for ch_t in c_half:
                nc.vector.memset(ch_t[:], 0.0)

            hs = []  # hs[t] = (h_lo, h_hi)
            hs_pool = tc.tile_pool(name="hspool", bufs=1)
            hsp = hs_pool.__enter__()

            def prev_slice(t, off, sz):
                if t == 0:
                    return hdT[:, off:off + sz]
                h_lo, h_hi = hs[t - 1]
                if off + sz <= HALVES[0][1]:
                    return h_lo[:, off:off + sz]
                o2 = off - HALVES[1][0]
                return h_hi[:, o2:o2 + sz]

            def ship_slice(i):
                c0, w = SL[i]
                nc.gpsimd.dma_start(
                    yshard_s[i][:].rearrange("p (k w) -> p k w", w=w),
                    ysb3[:].rearrange("p (k f) -> p k f", f=TNF)[:, :,
                                                                c0:c0 + w])
                nc.gpsimd.collective_compute(
                    "AllGather", mybir.AluOpType.bypass,
                    replica_groups=[list(range(NCORES))],
                    ins=[yshard_s[i].opt()], outs=[yfull_s[i].opt()],
                )
                ytab3 = ytab[:].rearrange("p (sb f) -> p sb f", f=TNF)
                for c in range(NCORES):
                    nc.gpsimd.dma_start(
                        ytab3[:, c * NTILES:(c + 1) * NTILES, c0:c0 + w],
                        yfull_s[i][c * P:(c + 1) * P, :].rearrange(
                            "p (k w) -> p k w", w=w))

            with tc.tile_pool(name="psG", bufs=2, space="PSUM") as psG, \
                 tc.tile_pool(name="psY", bufs=2, space="PSUM") as psY, \
                 tc.tile_pool(name="wpL", bufs=2) as wpL:

                def emit_proj(t):
                    h_lo, h_hi = hs[t]
                    pa_t = psY.tile([P, NTILES * NF], F32, tag="pa", bufs=2)
                    for k in range(NTILES):
                        rows = min(P, NCN - k * P)
                        if k * P < HALVES[0][1]:
                            lhsT = h_lo[:, k * P:k * P + rows]
                        else:
                            o2 = k * P - HALVES[1][0]
                            lhsT = h_hi[:, o2:o2 + rows]
                        nc.tensor.matmul(out=pa_t[:rows, k * NF:(k + 1) * NF],
                                         lhsT=lhsT, rhs=wcomb_sb[:],
                                         start=True, stop=True)
                    tcol = slice(t * NF, (t + 1) * NF)
                    ysr = ysb3[:].rearrange("p (k f) -> p k f", f=TNF)
                    nc.vector.tensor_copy(
                        out=ysr[:, 0:NTILES - 1, tcol],
                        in_=pa_t[:, :(NTILES - 1) * NF].rearrange(
                            "p (k f) -> p k f", f=NF))
                    lrows = NCN - (NTILES - 1) * P
                    nc.vector.tensor_copy(
                        out=ysr[:lrows, NTILES - 1:NTILES, tcol],
                        in_=pa_t[:lrows, (NTILES - 1) * NF:].rearrange(
                            "p (k f) -> p k f", f=NF))
                    if t in SHIP_T:
                        ship_slice(SHIP_T[t])

                for t in range(T):
                    sg = [None] * 4
                    pqs = [None] * 4

                    def emit_ih(q):
                        wsl = slice(q * H, (q + 1) * H)
                        pqs[q] = psG.tile([P, NCN], F32, name="pq", tag="pq", bufs=2)
                        for off, sz in CH:
                            nc.tensor.matmul(out=pqs[q][:, off:off + sz],
                                             lhsT=wih_sb[:, wsl],
                                             rhs=hdT[:, off:off + sz],
                                             start=True, stop=False)

                    def emit_hh_act(q):
                        wsl = slice(q * H, (q + 1) * H)
                        for off, sz in CH:
                            nc.tensor.matmul(out=pqs[q][:, off:off + sz],
                                             lhsT=whh_sb[:, wsl],
                                             rhs=prev_slice(t, off, sz),
                                             start=False, stop=True)
                        sg[q] = wpL.tile([P, NCN], F16, name=f"sg{q}",
                                         tag=f"sg{q}", bufs=2)
                        nc.scalar.activation(
                            out=sg[q][:], in_=pqs[q][:],
                            func=getattr(mybir.ActivationFunctionType,
                                         GATE_FUNCS[q]),
                            bias=bg_sb[:, q:q + 1])

                    emit_ih(GATE_ORDER[0])
                    emit_ih(GATE_ORDER[1])
                    if t > 0:
                        emit_proj(t - 1)   # fills PE while h_{t-1} finishes
                    emit_hh_act(GATE_ORDER[0])
                    emit_hh_act(GATE_ORDER[1])
                    for q in GATE_ORDER[2:]:
                        emit_ih(q)
                        emit_hh_act(q)

                    # tail: c = sig(f)*c + sig(i)*tanh(g); h = sig(o)*tanh(c)
                    # split into halves so h_lo lands early for step t+1.
                    for j, (off, sz) in enumerate(HALVES):
                        hsl = slice(off, off + sz)
                        nc.vector.tensor_mul(out=c_half[j][:],
                                             in0=c_half[j][:],
                                             in1=sg[1][:, hsl])
                        tmp = wpL.tile([P, sz], F16, tag=f"tmp{j}", bufs=2)
                        nc.vector.tensor_mul(out=tmp[:], in0=sg[0][:, hsl],
                                             in1=sg[2][:, hsl])
                        nc.vector.tensor_add(out=c_half[j][:],
                                             in0=c_half[j][:], in1=tmp[:])
                    h_t = [hsp.tile([P, sz], F16, name=f"h_{t}_{j}",
                                    tag=f"h_{t}_{j}")
                           for j, (off, sz) in enumerate(HALVES)]
                    for j, (off, sz) in enumerate(HALVES):
                        thc = wpL.tile([P, sz], F16, tag=f"thc{j}", bufs=2)
                        nc.scalar.activation(
                            out=thc[:], in_=c_half[j][:],
                            func=mybir.ActivationFunctionType.Tanh)
                        nc.vector.tensor_mul(out=h_t[j][:],
                                             in0=sg[3][:, off:off + sz],
                                             in1=thc[:])
                    hs.append(tuple(h_t))
                emit_proj(T - 1)
                ship_slice(3)
            hs_pool.__exit__(None, None, None)  # release hs SBUF before GCN

            # ---- GCN aggregation: agg[k] = sum_sb A[k,sb].T @ Y[sb] ----
            # Whole Y table SBUF-resident. A-blocks stream in 8-sb chunks.
            # 2 waves of 5 dst tiles.
            with tc.tile_pool(name="psC", bufs=1, space="PSUM") as psC, \
                 tc.tile_pool(name="wpO", bufs=2) as wpO:
                for w, wave in enumerate((range(0, 5), range(5, NTILES))):
                    wave = list(wave)
                    pa = {k: psC.tile([P, TNF], F32, name=f"pa_{k}",
                                      tag=f"pa{i}", bufs=1)
                          for i, k in enumerate(wave)}
                    for ci, (s0, s1) in enumerate(sb_chunks):
                        abc = abc_tiles[w * len(sb_chunks) + ci]
                        for sb in range(s0, s1):
                            aoff = (sb - s0) * 5 * P
                            for i, k in enumerate(wave):
                                nc.tensor.matmul(
                                    out=pa[k][:],
                                    lhsT=abc[:,
                                             aoff + i * P:aoff + (i + 1) * P],
                                    rhs=ytab[:,
                                             sb * TNF:(sb + 1) * TNF],
                                    start=(sb == 0),
                                    stop=(sb == NSB - 1))
                    for i, k in enumerate(wave):
                        rows = min(P, NCN - k * P)
                        osb = wpO.tile([P, TNF], F32, tag="osb", bufs=2)
                        nc.vector.tensor_add(out=osb[:rows], in0=pa[k][:rows],
                                             in1=bout_sb[:rows])
                        nc.sync.dma_start(xhat[k * P:k * P + rows, :],
                                          osb[:rows])
            wpC_pool.__exit__(None, None, None)

    nc.compile()
    return nc


def _preprocess(z, edge_index, x_mask, W_fc2, b_fc2, W_ih, W_hh, b_ih, b_hh,
                W_gcn, b_gcn, W_fc3, b_fc3):
    z = np.asarray(z, np.float32)
    edge_index = np.asarray(edge_index).astype(np.int64)
    x_mask = np.asarray(x_mask)
    src = edge_index[0]
    dst = edge_index[1]
    deg = (np.bincount(dst, minlength=N) + 1.0)
    dinv = (1.0 / np.sqrt(deg)).astype(np.float32)
    node_mask = x_mask.reshape(N, -1).any(axis=1).astype(np.float32)

    loop = np.arange(N, dtype=np.int64)
    src_all = np.concatenate([src, loop])
    dst_all = np.concatenate([dst, loop])
    val_all = dinv[src_all] * dinv[dst_all] * node_mask[src_all]

    # densify normalized adjacency into per-core wave-major A blocks:
    # row (w*NSB + sb)*128 + srel ; col (ktile%5)*128 + drel  (w = ktile//5)
    core_of = dst_all // NCN
    ktile = (dst_all % NCN) // P
    drel = (dst_all % NCN) % P
    src_pad = (src_all // NCN) * NPAD + (src_all % NCN)
    sblk = src_pad // P
    srel = src_pad % P

    a_blocks = []
    lin = (((ktile // 5) * NSB + sblk) * P + srel) * (5 * P) \
        + (ktile % 5) * P + drel
    nblk_lin = 2 * NSB * P * 5 * P
    for c in range(NCORES):
        m = core_of == c
        acc = np.bincount(lin[m], weights=val_all[m].astype(np.float64),
                          minlength=nblk_lin)
        a_blocks.append(acc.astype(np.float16).reshape(2 * NSB * P, 5 * P))

    Wcomb = np.ascontiguousarray((np.asarray(W_gcn, np.float32)
                                  @ np.asarray(W_fc3, np.float32))
                                 .astype(np.float16))
    bias16 = (np.asarray(b_gcn, np.float32) @ np.asarray(W_fc3, np.float32)
              + np.asarray(b_fc3, np.float32))
    bout_t = np.ascontiguousarray(np.tile(bias16, (P, T)).astype(np.float32))
    bgv = (np.asarray(b_ih, np.float32) + np.asarray(b_hh, np.float32))
    bg_t = np.ascontiguousarray(bgv.reshape(4, P).T.astype(np.float32))
    b2_t = np.ascontiguousarray(np.asarray(b_fc2, np.float32).reshape(P, 1))
    wih_t = np.ascontiguousarray(
        np.asarray(W_ih, np.float32).T.astype(np.float16))
    whh_t = np.ascontiguousarray(
        np.asarray(W_hh, np.float32).T.astype(np.float16))
    wfc2_t = np.ascontiguousarray(np.asarray(W_fc2, np.float32))

    in_maps = []
    for c in range(NCORES):
        sl = slice(c * NCN, (c + 1) * NCN)
        in_maps.append({
            "zT": np.ascontiguousarray(z[sl].T),
            "wfc2": wfc2_t,
            "b2": b2_t,
            "wih": wih_t,
            "whh": whh_t,
            "bg": bg_t,
            "wcomb": Wcomb,
            "bout": bout_t,
            "ablk": a_blocks[c],
        })
    return in_maps


def kernel(z, edge_index, x_mask, W_fc2, b_fc2, W_ih, W_hh, b_ih, b_hh,
           W_gcn, b_gcn, W_fc3, b_fc3):
    global LAST_RESULTS
    in_maps = _preprocess(z, edge_index, x_mask, W_fc2, b_fc2,
                          W_ih, W_hh, b_ih, b_hh,
                          W_gcn, b_gcn, W_fc3, b_fc3)
    if "nc" not in _BUILD_CACHE:
        _BUILD_CACHE["nc"] = _build()
    nc = _BUILD_CACHE["nc"]

    trace = bool(int(os.environ.get("KERNEL_TRACE", "0")))
    res = bass_utils.run_bass_kernel_spmd(
        nc, in_maps, core_ids=list(range(NCORES)), trace=trace)
    LAST_RESULTS = res

    out = np.empty((N, T, NF), np.float32)
    for c in range(NCORES):
        out[c * NCN:(c + 1) * NCN] = res.results[c]["xhat"].reshape(NCN, T,
                                                                    NF)
    return out
